# revision 21
# baseline (speedup 1.0000x reference)
"""Correlation-loss kernel for Trainium2 (8 NeuronCores, SPMD data-parallel).

Problem: for 800 random 16x16 patches of a 64-channel MSI image (first 32
channels used) and a 3-channel HE image, compute per-patch masked pairwise
squared-distance matrices over the 256 positions for both modalities and
L1-compare them; output sum(per-patch mean)/160.

Formulation: per patch, with mask m and sqd = sum_c msi^2 - sum_c he^2,
    out[a,b] = -(dm-dh)[a,b]/2 * m[a]m[b]
is a single rank-37 matmul lhsT.T @ rhs with
    lhsT = [xm*m (32) | -xh*m (3) | -sqd*m/2 | -m]   (K=37, cols=positions)
    rhs  = [xm*m (32) |  xh*m (3) |  m       | sqd*m/2]
and loss = sum_patches 2*sum|out| / 256^2 / 160 (abs kills the global sign).
out is symmetric, so only the upper 128-row chunk (D1|B, N=256) and the
lower-right diagonal block (D2, N=128) are computed. The double weight of
the off-diagonal block B is baked in on the host by doubling rhs columns
128:256 for the first matmul (a separate undoubled copy feeds the second),
so on-device everything is a single plain abs-sum taken straight out of
PSUM by ACT (Abs + accumulator) and DVE (abs-reduce) on disjoint patches
(= disjoint PSUM banks, keeping the engines parallel).

Memory layout: SBUF DMA bandwidth scales with the partition span of the
transfer (16 AXI ports x 8 partitions each), so 37-row operands are packed
two-per-128-partitions: even patches at partitions 0:37, odd at 64:101
(64 is the only legal matmul row offset for K=37), zeros between. DMAs
then run at full port width; odd-patch matmuls pass tile_position=(64,0).

Sharding: 100 patches per core, operands pre-gathered and bf16-cast on
host, partial sums returned per core, final scalar on host.
"""

import os
import sys

sys.path.insert(0, "/opt/trn_rl_repo")

import ml_dtypes
import numpy as np

import concourse.bass as bass  # noqa: F401
import concourse.tile as tile
from concourse import bacc, mybir
from concourse.bass_utils import run_bass_kernel_spmd

WS = 16
NB = 800
TH = 0.05
P = WS * WS  # 256
HP = P // 2  # 128
N_CORES = 8
PPC = NB // N_CORES  # 100
HPC = PPC // 2  # 50 patches per band
K = 37
BATCH = 2  # patches per PSUM sub-group (2 x 512 f32 = 2 banks; bufs=4)
NGROUP = PPC // BATCH  # 50
# input DMA chunk ladder (in half-patches; small first chunks let compute
# start while the bulk is still in flight). All chunks go down the single
# sync HWDGE ring in consumption order (mega[k] then rhsb[k] per segment).
DMA_LADDER = [1, 1, 2, 4, 8, 14, 20]

F32 = mybir.dt.float32
BF16 = mybir.dt.bfloat16

LAST_EXEC_NS = None
LAST_RESULTS = None

_compiled = None


def _build_program():
    nc = bacc.Bacc(
        "TRN2", target_bir_lowering=False, debug=False, num_devices=N_CORES
    )

    # mega: per half-patch h, cols [0:256)=lhs, [256:512)=rhsa(B cols doubled)
    # rows 0:37 even patches, 64:101 odd patches, zeros 37:64.
    # Only rows 0:101 are shipped (rows 101:128 of the tiles stay unwritten).
    NR = 64 + K  # 101
    mega_d = nc.dram_tensor("mega", [NR, HPC, 2 * P], BF16, kind="ExternalInput").ap()
    rhsb_d = nc.dram_tensor("rhsb", [NR, HPC, HP], BF16, kind="ExternalInput").ap()
    out_d = nc.dram_tensor("partial", [128, 1], F32, kind="ExternalOutput").ap()

    NSLOT = BATCH  # DVE sub-groups: one slot per patch; ACT sub-groups: 1 slot

    with tile.TileContext(nc) as tc:
        with (
            tc.tile_pool(name="ops", bufs=1) as opool,
            tc.tile_pool(name="psum", bufs=4, space="PSUM") as ppool,
            tc.tile_pool(name="accs", bufs=1) as apool,
            tc.tile_pool(name="scratch", bufs=4) as spool,
        ):
            mega = opool.tile([128, HPC, 2 * P], BF16)
            rhsb = opool.tile([128, HPC, HP], BF16)
            # mega on the sync HWDGE ring in consumption order; the small
            # rhsb chunks go down the scalar HWDGE ring (issued before any
            # ACTIVATE is ready, so they don't block compute). gpsimd SWDGE
            # transfers are far slower -- never use them for bulk.
            off = 0
            for w in DMA_LADDER:
                sl = slice(off, off + w)
                off += w
                nc.sync.dma_start(mega[0:NR, sl], mega_d[:, sl])
                nc.scalar.dma_start(rhsb[0:NR, sl], rhsb_d[:, sl])

            slots = apool.tile([128, NGROUP * NSLOT], F32)
            nc.vector.memset(slots[:], 0.0)

            for g in range(NGROUP):
                ps = ppool.tile([128, BATCH, 2 * P], F32)
                for pp in range(BATCH):
                    p = g * BATCH + pp
                    h = p // 2
                    if p % 2 == 0:
                        band = slice(0, K)
                        tp = None
                    else:
                        band = slice(64, 64 + K)
                        tp = (64, 0)
                    nc.tensor.matmul(
                        ps[:, pp, 0:P],
                        mega[band, h, 0:HP],
                        mega[band, h, P : 2 * P],
                        start=True,
                        stop=True,
                        tile_position=tp,
                    )
                    nc.tensor.matmul(
                        ps[:, pp, P : P + HP],
                        mega[band, h, HP:P],
                        rhsb[band, h, :],
                        start=True,
                        stop=True,
                        tile_position=tp,
                    )

                # per-patch psum cols 0:384 hold [D1 | 2B | D2]; plain abs-sum.
                # Sub-group goes to ONE engine; alternate engines every TWO
                # sub-groups so each engine sees back-to-back ops while the
                # other covers the next pair. Banks never shared.
                if (g // 2) % 2 == 0:
                    sc = spool.tile([128, BATCH, 3 * HP], F32, tag="sc")
                    nc.scalar.activation(
                        sc[:],
                        ps[:, :, 0 : 3 * HP],
                        mybir.ActivationFunctionType.Abs,
                        accum_out=slots[:, g * NSLOT : g * NSLOT + 1],
                    )
                else:
                    nc.vector.tensor_reduce(
                        slots[:, g * NSLOT : (g + 1) * NSLOT],
                        ps[:, :, 0 : 3 * HP],
                        axis=mybir.AxisListType.X,
                        op=mybir.AluOpType.add,
                        apply_absolute_value=True,
                    )

            out_t = apool.tile([128, 1], F32)
            nc.vector.tensor_reduce(
                out_t[:],
                slots[:].rearrange("q (a c) -> q a c", a=1),
                axis=mybir.AxisListType.XY,
                op=mybir.AluOpType.add,
            )
            nc.sync.dma_start(out_d[:], out_t[:])

    nc.compile()
    return nc


def _prep_operands(tensor_msi, tensor_he, i_idx, j_idx):
    """Host gather + operand build.

    Returns mega [N_CORES,128,HPC,2P] and rhsb [N_CORES,128,HPC,HP] bf16.
    """
    msi = np.ascontiguousarray(tensor_msi[0, :32], dtype=np.float32)
    he = np.ascontiguousarray(tensor_he[0], dtype=np.float32)
    ii = np.asarray(i_idx).astype(np.int64)
    jj = np.asarray(j_idx).astype(np.int64)

    ig = np.broadcast_to((ii[:, None] + np.arange(WS))[:, :, None], (NB, WS, WS))
    jg = np.broadcast_to((jj[:, None] + np.arange(WS))[:, None, :], (NB, WS, WS))
    pm = msi[:, ig, jg].transpose(1, 0, 2, 3).reshape(NB, 32, P)
    ph = he[:, ig, jg].transpose(1, 0, 2, 3).reshape(NB, 3, P)

    m = (ph.sum(axis=1) >= TH).astype(np.float32)
    sq = (pm * pm).sum(1) - (ph * ph).sum(1)
    pm_m = pm * m[:, None]
    ph_m = ph * m[:, None]
    sqm = (0.5 * sq * m)[:, None]
    mm = m[:, None]

    lhsT = np.concatenate([pm_m, -ph_m, -sqm, -mm], axis=1)  # [NB, K, P]
    rhs = np.concatenate([pm_m, ph_m, mm, sqm], axis=1)
    rhsa = rhs.copy()
    rhsa[:, :, HP:] *= 2.0
    rhsb = np.ascontiguousarray(rhs[:, :, HP:])

    lhsT = lhsT.reshape(N_CORES, PPC, K, P)
    rhsa = rhsa.reshape(N_CORES, PPC, K, P)
    rhsb = rhsb.reshape(N_CORES, PPC, K, HP)

    NR = 64 + K
    mega = np.zeros((N_CORES, NR, HPC, 2 * P), dtype=ml_dtypes.bfloat16)
    rb = np.zeros((N_CORES, NR, HPC, HP), dtype=ml_dtypes.bfloat16)
    for par, base in ((0, 0), (1, 64)):
        rows = slice(base, base + K)
        # [N_CORES, HPC, K, P] -> [N_CORES, K, HPC, P]
        mega[:, rows, :, 0:P] = (
            lhsT[:, par::2].transpose(0, 2, 1, 3).astype(ml_dtypes.bfloat16)
        )
        mega[:, rows, :, P : 2 * P] = (
            rhsa[:, par::2].transpose(0, 2, 1, 3).astype(ml_dtypes.bfloat16)
        )
        rb[:, rows] = (
            rhsb[:, par::2].transpose(0, 2, 1, 3).astype(ml_dtypes.bfloat16)
        )
    return np.ascontiguousarray(mega), np.ascontiguousarray(rb)


def kernel(tensor_msi, tensor_he, i_idx, j_idx, window_size, batch):
    global _compiled, LAST_EXEC_NS, LAST_RESULTS
    assert int(window_size) == WS and int(batch) == NB

    mega, rb = _prep_operands(
        np.asarray(tensor_msi), np.asarray(tensor_he), i_idx, j_idx
    )

    if _compiled is None:
        _compiled = _build_program()
    nc = _compiled

    in_maps = [{"mega": mega[c], "rhsb": rb[c]} for c in range(N_CORES)]

    trace = bool(os.environ.get("KERNEL_TRACE"))
    res = run_bass_kernel_spmd(
        nc, in_maps, core_ids=list(range(N_CORES)), trace=trace
    )
    LAST_EXEC_NS = res.exec_time_ns
    LAST_RESULTS = res

    total = np.float64(0.0)
    for c in range(N_CORES):
        total += res.results[c]["partial"].astype(np.float64).sum()
    loss = total * 2.0 / (P * P) / (NB // 5)
    return np.float32(loss)


# revision 25
# speedup vs baseline: 5.7026x; 5.7026x over previous
"""Correlation-loss kernel for Trainium2 (8 NeuronCores, SPMD data-parallel).

Problem: for 800 random 16x16 patches of a 64-channel MSI image (first 32
channels used) and a 3-channel HE image, compute per-patch masked pairwise
squared-distance matrices over the 256 positions for both modalities and
L1-compare them; output sum(per-patch mean)/160.

Formulation: per patch, with mask m and sqd = sum_c msi^2 - sum_c he^2,
    out[a,b] = -(dm-dh)[a,b]/2 * m[a]m[b]
is a single rank-37 matmul lhsT.T @ rhs with
    lhsT = [xm*m (32) | -xh*m (3) | -sqd*m/2 | -m]   (K=37, cols=positions)
    rhs  = [xm*m (32) |  xh*m (3) |  m       | sqd*m/2]
and loss = sum_patches 2*sum|out| / 256^2 / 160 (abs kills the global sign).
out is symmetric, so only the upper 128-row chunk (D1|B, N=256) and the
lower-right diagonal block (D2, N=128) are computed. The double weight of
the off-diagonal block B is baked in on the host by doubling rhs columns
128:256 for the first matmul (a separate undoubled copy feeds the second),
so on-device everything is a single plain abs-sum taken straight out of
PSUM by ACT (Abs + accumulator) and DVE (abs-reduce) on disjoint patches
(= disjoint PSUM banks, keeping the engines parallel).

Memory layout: SBUF DMA bandwidth scales with the partition span of the
transfer (16 AXI ports x 8 partitions each), so 37-row operands are packed
two-per-128-partitions: even patches at partitions 0:37, odd at 64:101
(64 is the only legal matmul row offset for K=37), zeros between. DMAs
then run at full port width; odd-patch matmuls pass tile_position=(64,0).

Sharding: 100 patches per core, operands pre-gathered and bf16-cast on
host, partial sums returned per core, final scalar on host.
"""

import os
import sys

sys.path.insert(0, "/opt/trn_rl_repo")

import ml_dtypes
import numpy as np

import concourse.bass as bass  # noqa: F401
import concourse.tile as tile
from concourse import bacc, mybir
from concourse.bass_utils import run_bass_kernel_spmd

WS = 16
NB = 800
TH = 0.05
P = WS * WS  # 256
HP = P // 2  # 128
N_CORES = 8
PPC = NB // N_CORES  # 100
HPC = PPC // 2  # 50 patches per band
K = 37
BATCH = 2  # patches per PSUM sub-group (2 x 512 f32 = 2 banks; bufs=4)
NGROUP = PPC // BATCH  # 50
# input DMA chunk ladders (in half-patches; small first chunks let compute
# start while the bulk is still in flight). mega goes down the sync HWDGE
# ring, rhsb down the scalar ring. DMAs MUST span all 128 partitions --
# any other span falls off the DIRECT2D fast path (measured 10-100x slower).
DMA_LADDER = [1, 1, 2, 3, 5, 6, 8, 8, 8, 8]
RB_LADDER = [4, 10, 16, 20]

F32 = mybir.dt.float32
BF16 = mybir.dt.bfloat16

LAST_EXEC_NS = None
LAST_RESULTS = None

_compiled = None


def _build_program():
    nc = bacc.Bacc(
        "TRN2", target_bir_lowering=False, debug=False, num_devices=N_CORES
    )

    # mega: per half-patch h, cols [0:256)=lhs, [256:512)=rhsa(B cols doubled)
    # rows 0:37 even patches, 64:101 odd patches, zeros elsewhere
    mega_d = nc.dram_tensor("mega", [128, HPC, 2 * P], BF16, kind="ExternalInput").ap()
    rhsb_d = nc.dram_tensor("rhsb", [128, HPC, HP], BF16, kind="ExternalInput").ap()
    out_d = nc.dram_tensor("partial", [128, 1], F32, kind="ExternalOutput").ap()

    NSLOT = BATCH  # DVE sub-groups: one slot per patch; ACT sub-groups: 1 slot

    with tile.TileContext(nc) as tc:
        with (
            tc.tile_pool(name="ops", bufs=1) as opool,
            tc.tile_pool(name="psum", bufs=4, space="PSUM") as ppool,
            tc.tile_pool(name="accs", bufs=1) as apool,
            tc.tile_pool(name="scratch", bufs=4) as spool,
        ):
            mega = opool.tile([128, HPC, 2 * P], BF16)
            rhsb = opool.tile([128, HPC, HP], BF16)
            # mega on the sync HWDGE ring in consumption order; the small
            # rhsb chunks go down the scalar HWDGE ring (issued before any
            # ACTIVATE is ready, so they don't block compute). gpsimd SWDGE
            # transfers are far slower -- never use them for bulk.
            off = 0
            for w in DMA_LADDER:
                sl = slice(off, off + w)
                off += w
                nc.sync.dma_start(mega[:, sl], mega_d[:, sl])
            off = 0
            for w in RB_LADDER:
                sl = slice(off, off + w)
                off += w
                nc.scalar.dma_start(rhsb[:, sl], rhsb_d[:, sl])

            slots = apool.tile([128, NGROUP * NSLOT], F32)
            nc.vector.memset(slots[:], 0.0)

            for g in range(NGROUP):
                ps = ppool.tile([128, BATCH, 2 * P], F32)
                for pp in range(BATCH):
                    p = g * BATCH + pp
                    h = p // 2
                    if p % 2 == 0:
                        band = slice(0, K)
                        tp = None
                    else:
                        band = slice(64, 64 + K)
                        tp = (64, 0)
                    nc.tensor.matmul(
                        ps[:, pp, 0:P],
                        mega[band, h, 0:HP],
                        mega[band, h, P : 2 * P],
                        start=True,
                        stop=True,
                        tile_position=tp,
                    )
                    nc.tensor.matmul(
                        ps[:, pp, P : P + HP],
                        mega[band, h, HP:P],
                        rhsb[band, h, :],
                        start=True,
                        stop=True,
                        tile_position=tp,
                    )

                # per-patch psum cols 0:384 hold [D1 | 2B | D2]; plain abs-sum.
                # Sub-group goes to ONE engine; alternate engines every TWO
                # sub-groups so each engine sees back-to-back ops while the
                # other covers the next pair. Banks never shared.
                if (g // 2) % 2 == 0:
                    sc = spool.tile([128, BATCH, 3 * HP], F32, tag="sc")
                    nc.scalar.activation(
                        sc[:],
                        ps[:, :, 0 : 3 * HP],
                        mybir.ActivationFunctionType.Abs,
                        accum_out=slots[:, g * NSLOT : g * NSLOT + 1],
                    )
                else:
                    nc.vector.tensor_reduce(
                        slots[:, g * NSLOT : (g + 1) * NSLOT],
                        ps[:, :, 0 : 3 * HP],
                        axis=mybir.AxisListType.X,
                        op=mybir.AluOpType.add,
                        apply_absolute_value=True,
                    )

            out_t = apool.tile([128, 1], F32)
            nc.vector.tensor_reduce(
                out_t[:],
                slots[:].rearrange("q (a c) -> q a c", a=1),
                axis=mybir.AxisListType.XY,
                op=mybir.AluOpType.add,
            )
            nc.sync.dma_start(out_d[:], out_t[:])

    nc.compile()
    return nc


def _prep_operands(tensor_msi, tensor_he, i_idx, j_idx):
    """Host gather + operand build.

    Returns mega [N_CORES,128,HPC,2P] and rhsb [N_CORES,128,HPC,HP] bf16.
    """
    msi = np.ascontiguousarray(tensor_msi[0, :32], dtype=np.float32)
    he = np.ascontiguousarray(tensor_he[0], dtype=np.float32)
    ii = np.asarray(i_idx).astype(np.int64)
    jj = np.asarray(j_idx).astype(np.int64)

    ig = np.broadcast_to((ii[:, None] + np.arange(WS))[:, :, None], (NB, WS, WS))
    jg = np.broadcast_to((jj[:, None] + np.arange(WS))[:, None, :], (NB, WS, WS))
    pm = msi[:, ig, jg].transpose(1, 0, 2, 3).reshape(NB, 32, P)
    ph = he[:, ig, jg].transpose(1, 0, 2, 3).reshape(NB, 3, P)

    m = (ph.sum(axis=1) >= TH).astype(np.float32)
    sq = (pm * pm).sum(1) - (ph * ph).sum(1)
    pm_m = pm * m[:, None]
    ph_m = ph * m[:, None]
    sqm = (0.5 * sq * m)[:, None]
    mm = m[:, None]

    lhsT = np.concatenate([pm_m, -ph_m, -sqm, -mm], axis=1)  # [NB, K, P]
    rhs = np.concatenate([pm_m, ph_m, mm, sqm], axis=1)
    rhsa = rhs.copy()
    rhsa[:, :, HP:] *= 2.0
    rhsb = np.ascontiguousarray(rhs[:, :, HP:])

    lhsT = lhsT.reshape(N_CORES, PPC, K, P)
    rhsa = rhsa.reshape(N_CORES, PPC, K, P)
    rhsb = rhsb.reshape(N_CORES, PPC, K, HP)

    mega = np.zeros((N_CORES, 128, HPC, 2 * P), dtype=ml_dtypes.bfloat16)
    rb = np.zeros((N_CORES, 128, HPC, HP), dtype=ml_dtypes.bfloat16)
    for par, base in ((0, 0), (1, 64)):
        rows = slice(base, base + K)
        # [N_CORES, HPC, K, P] -> [N_CORES, K, HPC, P]
        mega[:, rows, :, 0:P] = (
            lhsT[:, par::2].transpose(0, 2, 1, 3).astype(ml_dtypes.bfloat16)
        )
        mega[:, rows, :, P : 2 * P] = (
            rhsa[:, par::2].transpose(0, 2, 1, 3).astype(ml_dtypes.bfloat16)
        )
        rb[:, rows] = (
            rhsb[:, par::2].transpose(0, 2, 1, 3).astype(ml_dtypes.bfloat16)
        )
    return np.ascontiguousarray(mega), np.ascontiguousarray(rb)


def kernel(tensor_msi, tensor_he, i_idx, j_idx, window_size, batch):
    global _compiled, LAST_EXEC_NS, LAST_RESULTS
    assert int(window_size) == WS and int(batch) == NB

    mega, rb = _prep_operands(
        np.asarray(tensor_msi), np.asarray(tensor_he), i_idx, j_idx
    )

    if _compiled is None:
        _compiled = _build_program()
    nc = _compiled

    in_maps = [{"mega": mega[c], "rhsb": rb[c]} for c in range(N_CORES)]

    trace = bool(os.environ.get("KERNEL_TRACE"))
    res = run_bass_kernel_spmd(
        nc, in_maps, core_ids=list(range(N_CORES)), trace=trace
    )
    LAST_EXEC_NS = res.exec_time_ns
    LAST_RESULTS = res

    total = np.float64(0.0)
    for c in range(N_CORES):
        total += res.results[c]["partial"].astype(np.float64).sum()
    loss = total * 2.0 / (P * P) / (NB // 5)
    return np.float32(loss)


# revision 27
# speedup vs baseline: 5.9615x; 1.0454x over previous
"""Correlation-loss kernel for Trainium2 (8 NeuronCores, SPMD data-parallel).

Problem: for 800 random 16x16 patches of a 64-channel MSI image (first 32
channels used) and a 3-channel HE image, compute per-patch masked pairwise
squared-distance matrices over the 256 positions for both modalities and
L1-compare them; output sum(per-patch mean)/160.

Formulation: per patch, with mask m and sqd = sum_c msi^2 - sum_c he^2,
    out[a,b] = -(dm-dh)[a,b]/2 * m[a]m[b]
is a single rank-37 matmul lhsT.T @ rhs with
    lhsT = [xm*m (32) | -xh*m (3) | -sqd*m/2 | -m]   (K=37, cols=positions)
    rhs  = [xm*m (32) |  xh*m (3) |  m       | sqd*m/2]
and loss = sum_patches 2*sum|out| / 256^2 / 160 (abs kills the global sign).
out is symmetric, so only the upper 128-row chunk (D1|B, N=256) and the
lower-right diagonal block (D2, N=128) are computed. The double weight of
the off-diagonal block B is baked in on the host by doubling rhs columns
128:256 for the first matmul (a separate undoubled copy feeds the second),
so on-device everything is a single plain abs-sum taken straight out of
PSUM by ACT (Abs + accumulator) and DVE (abs-reduce) on disjoint patches
(= disjoint PSUM banks, keeping the engines parallel).

Memory layout: SBUF DMA bandwidth scales with the partition span of the
transfer (16 AXI ports x 8 partitions each), so 37-row operands are packed
two-per-128-partitions: even patches at partitions 0:37, odd at 64:101
(64 is the only legal matmul row offset for K=37), zeros between. DMAs
then run at full port width; odd-patch matmuls pass tile_position=(64,0).

Sharding: 100 patches per core, operands pre-gathered and bf16-cast on
host, partial sums returned per core, final scalar on host.
"""

import os
import sys

sys.path.insert(0, "/opt/trn_rl_repo")

import ml_dtypes
import numpy as np

import concourse.bass as bass  # noqa: F401
import concourse.tile as tile
from concourse import bacc, mybir
from concourse.bass_utils import run_bass_kernel_spmd

WS = 16
NB = 800
TH = 0.05
P = WS * WS  # 256
HP = P // 2  # 128
N_CORES = 8
PPC = NB // N_CORES  # 100
HPC = PPC // 2  # 50 patches per band
K = 37
BATCH = 2  # patches per PSUM sub-group (2 x 512 f32 = 2 banks; bufs=4)
NGROUP = PPC // BATCH  # 50
# input DMA chunk ladders (in half-patches; small first chunks let compute
# start while the bulk is still in flight). mega goes down the sync HWDGE
# ring, rhsb down the scalar ring. DMAs MUST span all 128 partitions --
# any other span falls off the DIRECT2D fast path (measured 10-100x slower).
DMA_LADDER = [1, 1, 2, 3, 5, 6, 8, 8, 8, 8]
RB_LADDER = [4, 10, 16, 20]

F32 = mybir.dt.float32
BF16 = mybir.dt.bfloat16

LAST_EXEC_NS = None
LAST_RESULTS = None

_compiled = None


def _build_program():
    nc = bacc.Bacc(
        "TRN2", target_bir_lowering=False, debug=False, num_devices=N_CORES
    )

    # mega: per half-patch h, cols [0:256)=lhs, [256:512)=rhsa(B cols doubled)
    # rows 0:37 even patches, 64:101 odd patches, zeros elsewhere
    mega_d = nc.dram_tensor("mega", [128, HPC, 2 * P], BF16, kind="ExternalInput").ap()
    rhsb_d = nc.dram_tensor("rhsb", [128, HPC, HP], BF16, kind="ExternalInput").ap()
    out_d = nc.dram_tensor("partial", [128, 1], F32, kind="ExternalOutput").ap()

    NSLOT = BATCH  # DVE sub-groups: one slot per patch; ACT sub-groups: 1 slot

    with tile.TileContext(nc) as tc:
        with (
            tc.tile_pool(name="ops", bufs=1) as opool,
            tc.tile_pool(name="psum", bufs=4, space="PSUM") as ppool,
            tc.tile_pool(name="accs", bufs=1) as apool,
            tc.tile_pool(name="scratch", bufs=4) as spool,
        ):
            mega = opool.tile([128, HPC, 2 * P], BF16)
            rhsb = opool.tile([128, HPC, HP], BF16)
            # mega on the sync HWDGE ring in consumption order; the small
            # rhsb chunks go down the scalar HWDGE ring (issued before any
            # ACTIVATE is ready, so they don't block compute). gpsimd SWDGE
            # transfers are far slower -- never use them for bulk.
            off = 0
            for w in DMA_LADDER:
                sl = slice(off, off + w)
                off += w
                nc.sync.dma_start(mega[:, sl], mega_d[:, sl])
            off = 0
            for w in RB_LADDER:
                sl = slice(off, off + w)
                off += w
                nc.scalar.dma_start(rhsb[:, sl], rhsb_d[:, sl])

            slots = apool.tile([128, NGROUP * NSLOT], F32)
            nc.vector.memset(slots[:], 0.0)
            zbias = apool.tile([128, 1], F32)
            nc.vector.memset(zbias[:], 0.0)

            for g in range(NGROUP):
                ps = ppool.tile([128, BATCH, 2 * P], F32)
                for pp in range(BATCH):
                    p = g * BATCH + pp
                    h = p // 2
                    if p % 2 == 0:
                        band = slice(0, K)
                        tp = None
                    else:
                        band = slice(64, 64 + K)
                        tp = (64, 0)
                    nc.tensor.matmul(
                        ps[:, pp, 0:P],
                        mega[band, h, 0:HP],
                        mega[band, h, P : 2 * P],
                        start=True,
                        stop=True,
                        tile_position=tp,
                    )
                    nc.tensor.matmul(
                        ps[:, pp, P : P + HP],
                        mega[band, h, HP:P],
                        rhsb[band, h, :],
                        start=True,
                        stop=True,
                        tile_position=tp,
                    )

                # per-patch psum cols 0:384 hold [D1 | 2B | D2]; plain abs-sum.
                # Sub-groups alternate engines (ABAB): each engine gets two
                # sub-group periods per op, the pair always overlaps. Banks
                # never shared between engines.
                if g % 2 == 0:
                    sc = spool.tile([128, BATCH, 3 * HP], F32, tag="sc")
                    nc.scalar.activation(
                        sc[:],
                        ps[:, :, 0 : 3 * HP],
                        mybir.ActivationFunctionType.Abs,
                        bias=zbias[:, 0:1],
                        accum_out=slots[:, g * NSLOT : g * NSLOT + 1],
                    )
                else:
                    nc.vector.tensor_reduce(
                        slots[:, g * NSLOT : (g + 1) * NSLOT],
                        ps[:, :, 0 : 3 * HP],
                        axis=mybir.AxisListType.X,
                        op=mybir.AluOpType.add,
                        apply_absolute_value=True,
                    )

            out_t = apool.tile([128, 1], F32)
            nc.vector.tensor_reduce(
                out_t[:],
                slots[:].rearrange("q (a c) -> q a c", a=1),
                axis=mybir.AxisListType.XY,
                op=mybir.AluOpType.add,
            )
            nc.sync.dma_start(out_d[:], out_t[:])

    nc.compile()
    return nc


def _prep_operands(tensor_msi, tensor_he, i_idx, j_idx):
    """Host gather + operand build.

    Returns mega [N_CORES,128,HPC,2P] and rhsb [N_CORES,128,HPC,HP] bf16.
    """
    msi = np.ascontiguousarray(tensor_msi[0, :32], dtype=np.float32)
    he = np.ascontiguousarray(tensor_he[0], dtype=np.float32)
    ii = np.asarray(i_idx).astype(np.int64)
    jj = np.asarray(j_idx).astype(np.int64)

    ig = np.broadcast_to((ii[:, None] + np.arange(WS))[:, :, None], (NB, WS, WS))
    jg = np.broadcast_to((jj[:, None] + np.arange(WS))[:, None, :], (NB, WS, WS))
    pm = msi[:, ig, jg].transpose(1, 0, 2, 3).reshape(NB, 32, P)
    ph = he[:, ig, jg].transpose(1, 0, 2, 3).reshape(NB, 3, P)

    m = (ph.sum(axis=1) >= TH).astype(np.float32)
    sq = (pm * pm).sum(1) - (ph * ph).sum(1)
    pm_m = pm * m[:, None]
    ph_m = ph * m[:, None]
    sqm = (0.5 * sq * m)[:, None]
    mm = m[:, None]

    lhsT = np.concatenate([pm_m, -ph_m, -sqm, -mm], axis=1)  # [NB, K, P]
    rhs = np.concatenate([pm_m, ph_m, mm, sqm], axis=1)
    rhsa = rhs.copy()
    rhsa[:, :, HP:] *= 2.0
    rhsb = np.ascontiguousarray(rhs[:, :, HP:])

    lhsT = lhsT.reshape(N_CORES, PPC, K, P)
    rhsa = rhsa.reshape(N_CORES, PPC, K, P)
    rhsb = rhsb.reshape(N_CORES, PPC, K, HP)

    mega = np.zeros((N_CORES, 128, HPC, 2 * P), dtype=ml_dtypes.bfloat16)
    rb = np.zeros((N_CORES, 128, HPC, HP), dtype=ml_dtypes.bfloat16)
    for par, base in ((0, 0), (1, 64)):
        rows = slice(base, base + K)
        # [N_CORES, HPC, K, P] -> [N_CORES, K, HPC, P]
        mega[:, rows, :, 0:P] = (
            lhsT[:, par::2].transpose(0, 2, 1, 3).astype(ml_dtypes.bfloat16)
        )
        mega[:, rows, :, P : 2 * P] = (
            rhsa[:, par::2].transpose(0, 2, 1, 3).astype(ml_dtypes.bfloat16)
        )
        rb[:, rows] = (
            rhsb[:, par::2].transpose(0, 2, 1, 3).astype(ml_dtypes.bfloat16)
        )
    return np.ascontiguousarray(mega), np.ascontiguousarray(rb)


def kernel(tensor_msi, tensor_he, i_idx, j_idx, window_size, batch):
    global _compiled, LAST_EXEC_NS, LAST_RESULTS
    assert int(window_size) == WS and int(batch) == NB

    mega, rb = _prep_operands(
        np.asarray(tensor_msi), np.asarray(tensor_he), i_idx, j_idx
    )

    if _compiled is None:
        _compiled = _build_program()
    nc = _compiled

    in_maps = [{"mega": mega[c], "rhsb": rb[c]} for c in range(N_CORES)]

    trace = bool(os.environ.get("KERNEL_TRACE"))
    res = run_bass_kernel_spmd(
        nc, in_maps, core_ids=list(range(N_CORES)), trace=trace
    )
    LAST_EXEC_NS = res.exec_time_ns
    LAST_RESULTS = res

    total = np.float64(0.0)
    for c in range(N_CORES):
        total += res.results[c]["partial"].astype(np.float64).sum()
    loss = total * 2.0 / (P * P) / (NB // 5)
    return np.float32(loss)


# revision 31
# speedup vs baseline: 6.1417x; 1.0302x over previous
"""Correlation-loss kernel for Trainium2 (8 NeuronCores, SPMD data-parallel).

Problem: for 800 random 16x16 patches of a 64-channel MSI image (first 32
channels used) and a 3-channel HE image, compute per-patch masked pairwise
squared-distance matrices over the 256 positions for both modalities and
L1-compare them; output sum(per-patch mean)/160.

Formulation: per patch, with mask m and sqd = sum_c msi^2 - sum_c he^2,
    out[a,b] = -(dm-dh)[a,b]/2 * m[a]m[b]
is a single rank-37 matmul lhsT.T @ rhs with
    lhsT = [xm*m (32) | -xh*m (3) | -sqd*m/2 | -m]   (K=37, cols=positions)
    rhs  = [xm*m (32) |  xh*m (3) |  m       | sqd*m/2]
and loss = sum_patches 2*sum|out| / 256^2 / 160 (abs kills the global sign).
out is symmetric, so only the upper 128-row chunk (D1|B, N=256) and the
lower-right diagonal block (D2, N=128) are computed. The double weight of
the off-diagonal block B is baked in on the host by doubling rhs columns
128:256 for the first matmul (a separate undoubled copy feeds the second),
so on-device everything is a single plain abs-sum taken straight out of
PSUM by ACT (Abs + accumulator) and DVE (abs-reduce) on disjoint patches
(= disjoint PSUM banks, keeping the engines parallel).

Memory layout: SBUF DMA bandwidth scales with the partition span of the
transfer (16 AXI ports x 8 partitions each), so 37-row operands are packed
two-per-128-partitions: even patches at partitions 0:37, odd at 64:101
(64 is the only legal matmul row offset for K=37), zeros between. DMAs
then run at full port width; odd-patch matmuls pass tile_position=(64,0).

Sharding: 100 patches per core, operands pre-gathered and bf16-cast on
host, partial sums returned per core, final scalar on host.
"""

import os
import sys

sys.path.insert(0, "/opt/trn_rl_repo")

import ml_dtypes
import numpy as np

import concourse.bass as bass  # noqa: F401
import concourse.tile as tile
from concourse import bacc, mybir
from concourse.bass_utils import run_bass_kernel_spmd

WS = 16
NB = 800
TH = 0.05
P = WS * WS  # 256
HP = P // 2  # 128
N_CORES = 8
PPC = NB // N_CORES  # 100
HPC = PPC // 2  # 50 patches per band
K = 37
BATCH = 2  # patches per PSUM sub-group (2 x 512 f32 = 2 banks; bufs=4)
NGROUP = PPC // BATCH  # 50
# input DMA chunk ladders (in half-patches; small first chunks let compute
# start while the bulk is still in flight). mega goes down the sync HWDGE
# ring, rhsb down the scalar ring. DMAs MUST span all 128 partitions --
# any other span falls off the DIRECT2D fast path (measured 10-100x slower).
DMA_LADDER = [1, 1, 2, 3, 5, 6, 8, 8, 8, 8]
RB_LADDER = [4, 10, 16, 20]

F32 = mybir.dt.float32
BF16 = mybir.dt.bfloat16

LAST_EXEC_NS = None
LAST_RESULTS = None

_compiled = None


def _build_program():
    nc = bacc.Bacc(
        "TRN2", target_bir_lowering=False, debug=False, num_devices=N_CORES
    )

    # mega: per half-patch h, cols [0:256)=lhs, [256:512)=rhsa(B cols doubled)
    # rows 0:37 even patches, 64:101 odd patches, zeros elsewhere
    mega_d = nc.dram_tensor("mega", [128, HPC, 2 * P], BF16, kind="ExternalInput").ap()
    rhsb_d = nc.dram_tensor("rhsb", [128, HPC, HP], BF16, kind="ExternalInput").ap()
    out_d = nc.dram_tensor("partial", [128, 2], F32, kind="ExternalOutput").ap()

    NSLOT = BATCH  # DVE sub-groups: one slot per patch; ACT sub-groups: 1 slot

    with tile.TileContext(nc) as tc:
        with (
            tc.tile_pool(name="ops", bufs=1) as opool,
            tc.tile_pool(name="psum", bufs=4, space="PSUM") as ppool,
            tc.tile_pool(name="accs", bufs=1) as apool,
            tc.tile_pool(name="scratch", bufs=4) as spool,
        ):
            mega = opool.tile([128, HPC, 2 * P], BF16)
            rhsb = opool.tile([128, HPC, HP], BF16)
            # mega on the sync HWDGE ring in consumption order; the small
            # rhsb chunks go down the scalar HWDGE ring (issued before any
            # ACTIVATE is ready, so they don't block compute). gpsimd SWDGE
            # transfers are far slower -- never use them for bulk.
            off = 0
            for w in DMA_LADDER:
                sl = slice(off, off + w)
                off += w
                nc.sync.dma_start(mega[:, sl], mega_d[:, sl])
            off = 0
            for w in RB_LADDER:
                sl = slice(off, off + w)
                off += w
                nc.scalar.dma_start(rhsb[:, sl], rhsb_d[:, sl])

            # separate slot tiles per engine: a shared tile would thread a
            # WAW dependency between every ACT and DVE op
            slots_a = apool.tile([128, NGROUP], F32)
            nc.vector.memset(slots_a[:], 0.0)
            slots_d = apool.tile([128, NGROUP * NSLOT], F32)
            nc.vector.memset(slots_d[:], 0.0)
            zbias = apool.tile([128, 1], F32)
            nc.vector.memset(zbias[:], 0.0)

            for g in range(NGROUP):
                ps = ppool.tile([128, BATCH, 2 * P], F32)
                for pp in range(BATCH):
                    p = g * BATCH + pp
                    h = p // 2
                    if p % 2 == 0:
                        band = slice(0, K)
                        tp = None
                    else:
                        band = slice(64, 64 + K)
                        tp = (64, 0)
                    nc.tensor.matmul(
                        ps[:, pp, 0:P],
                        mega[band, h, 0:HP],
                        mega[band, h, P : 2 * P],
                        start=True,
                        stop=True,
                        tile_position=tp,
                    )
                    nc.tensor.matmul(
                        ps[:, pp, P : P + HP],
                        mega[band, h, HP:P],
                        rhsb[band, h, :],
                        start=True,
                        stop=True,
                        tile_position=tp,
                    )

                # per-patch psum cols 0:384 hold [D1 | 2B | D2]; plain abs-sum.
                # Sub-groups alternate engines (ABAB): each engine gets two
                # sub-group periods per op, the pair always overlaps. Banks
                # never shared between engines.
                if g % 2 == 0:
                    sc = spool.tile([128, BATCH, 3 * HP], F32, tag="sc")
                    nc.scalar.activation(
                        sc[:],
                        ps[:, :, 0 : 3 * HP],
                        mybir.ActivationFunctionType.Abs,
                        bias=zbias[:, 0:1],
                        accum_out=slots_a[:, g : g + 1],
                    )
                else:
                    nc.vector.tensor_reduce(
                        slots_d[:, g * NSLOT : (g + 1) * NSLOT],
                        ps[:, :, 0 : 3 * HP],
                        axis=mybir.AxisListType.X,
                        op=mybir.AluOpType.add,
                        apply_absolute_value=True,
                    )

            out_t = apool.tile([128, 2], F32)
            nc.vector.tensor_reduce(
                out_t[:, 0:1],
                slots_a[:].rearrange("q (a c) -> q a c", a=1),
                axis=mybir.AxisListType.XY,
                op=mybir.AluOpType.add,
            )
            nc.vector.tensor_reduce(
                out_t[:, 1:2],
                slots_d[:].rearrange("q (a c) -> q a c", a=1),
                axis=mybir.AxisListType.XY,
                op=mybir.AluOpType.add,
            )
            nc.sync.dma_start(out_d[:], out_t[:])

    nc.compile()
    return nc


def _prep_operands(tensor_msi, tensor_he, i_idx, j_idx):
    """Host gather + operand build.

    Returns mega [N_CORES,128,HPC,2P] and rhsb [N_CORES,128,HPC,HP] bf16.
    """
    msi = np.ascontiguousarray(tensor_msi[0, :32], dtype=np.float32)
    he = np.ascontiguousarray(tensor_he[0], dtype=np.float32)
    ii = np.asarray(i_idx).astype(np.int64)
    jj = np.asarray(j_idx).astype(np.int64)

    ig = np.broadcast_to((ii[:, None] + np.arange(WS))[:, :, None], (NB, WS, WS))
    jg = np.broadcast_to((jj[:, None] + np.arange(WS))[:, None, :], (NB, WS, WS))
    pm = msi[:, ig, jg].transpose(1, 0, 2, 3).reshape(NB, 32, P)
    ph = he[:, ig, jg].transpose(1, 0, 2, 3).reshape(NB, 3, P)

    m = (ph.sum(axis=1) >= TH).astype(np.float32)
    sq = (pm * pm).sum(1) - (ph * ph).sum(1)
    pm_m = pm * m[:, None]
    ph_m = ph * m[:, None]
    sqm = (0.5 * sq * m)[:, None]
    mm = m[:, None]

    lhsT = np.concatenate([pm_m, -ph_m, -sqm, -mm], axis=1)  # [NB, K, P]
    rhs = np.concatenate([pm_m, ph_m, mm, sqm], axis=1)
    rhsa = rhs.copy()
    rhsa[:, :, HP:] *= 2.0
    rhsb = np.ascontiguousarray(rhs[:, :, HP:])

    lhsT = lhsT.reshape(N_CORES, PPC, K, P)
    rhsa = rhsa.reshape(N_CORES, PPC, K, P)
    rhsb = rhsb.reshape(N_CORES, PPC, K, HP)

    mega = np.zeros((N_CORES, 128, HPC, 2 * P), dtype=ml_dtypes.bfloat16)
    rb = np.zeros((N_CORES, 128, HPC, HP), dtype=ml_dtypes.bfloat16)
    for par, base in ((0, 0), (1, 64)):
        rows = slice(base, base + K)
        # [N_CORES, HPC, K, P] -> [N_CORES, K, HPC, P]
        mega[:, rows, :, 0:P] = (
            lhsT[:, par::2].transpose(0, 2, 1, 3).astype(ml_dtypes.bfloat16)
        )
        mega[:, rows, :, P : 2 * P] = (
            rhsa[:, par::2].transpose(0, 2, 1, 3).astype(ml_dtypes.bfloat16)
        )
        rb[:, rows] = (
            rhsb[:, par::2].transpose(0, 2, 1, 3).astype(ml_dtypes.bfloat16)
        )
    return np.ascontiguousarray(mega), np.ascontiguousarray(rb)


def kernel(tensor_msi, tensor_he, i_idx, j_idx, window_size, batch):
    global _compiled, LAST_EXEC_NS, LAST_RESULTS
    assert int(window_size) == WS and int(batch) == NB

    mega, rb = _prep_operands(
        np.asarray(tensor_msi), np.asarray(tensor_he), i_idx, j_idx
    )

    if _compiled is None:
        _compiled = _build_program()
    nc = _compiled

    in_maps = [{"mega": mega[c], "rhsb": rb[c]} for c in range(N_CORES)]

    trace = bool(os.environ.get("KERNEL_TRACE"))
    res = run_bass_kernel_spmd(
        nc, in_maps, core_ids=list(range(N_CORES)), trace=trace
    )
    LAST_EXEC_NS = res.exec_time_ns
    LAST_RESULTS = res

    total = np.float64(0.0)
    for c in range(N_CORES):
        total += res.results[c]["partial"].astype(np.float64).sum()
    loss = total * 2.0 / (P * P) / (NB // 5)
    return np.float32(loss)
